# revision 15
# baseline (speedup 1.0000x reference)
"""Multi-head attention (b=2, n=2048, d=1024, H=16 heads) on 8 TRN2 NeuronCores.

Sharding: core c = (b, g) with b = c // 4 (data parallel over batch) and
g = c % 4 (tensor parallel over head groups of 4 heads).  Each core computes
qkv projections for its 4 heads, full softmax attention for those heads, and
a partial output projection y_partial = A_heads @ w_out[g*256:(g+1)*256].
The host sums the 4 partials per batch and adds b_out.

Layout strategy (per core):
  - host passes xT = x[b].T  [1024, 2048] in bf16 (d on partitions when tiled)
  - qT, kT computed as [256, 2048] (head_dim*heads on partitions) via
    matmul(lhsT=w_slice, rhs=xT); head pairs (2m, 2m+1) sit on partition
    halves of m-tile m so the two K=64 score matmuls run concurrently in
    disjoint PE row groups
  - v computed as [2048, 256] via matmul(lhsT=xT, rhs=wv), stored per-head
    with an appended ones column (v_aug [nk, 65]) so the PV matmul also
    accumulates the softmax denominator
  - scores computed TRANSPOSED: ST[nk, nq] = matmul(lhsT=kT, rhs=qT); the two
    heads of a pair share one 2-bank PSUM tile so a single ACTIVATE exps 1024
    elements (ScalarE is the second-busiest engine; its per-op overhead is
    ~352 cycles)
  - softmax needs no row-max subtraction (scores ~ N(0,1), exp <= ~3e3)
  - PV: outT[65, nq] += matmul(lhsT=v_aug, rhs=exp(ST))
  - accumulators are staged to SBUF immediately (frees PSUM for the next
    pair) and normalized there: DVE reciprocal + gpsimd partition_broadcast
  - output projection for chunk c is emitted interleaved into the NEXT
    pair-block's inner loop so the PE never waits on the normalization chain
  - the first attention block is interleaved with the v projection so ScalarE
    starts exp work early
Matmuls run in bf16 (fp32 PSUM accumulation); measured end-to-end relative
error ~5e-3 vs the fp32 reference.
"""

import os
import sys

for _p in ("/opt/trn_rl_repo",):
    if _p not in sys.path and os.path.isdir(_p):
        sys.path.insert(0, _p)

import ml_dtypes
import numpy as np

import concourse.bass as bass
import concourse.mybir as mybir
import concourse.tile as tile
from concourse import bacc

P = 128
D = 1024          # model dim
N = 2048          # sequence length
HD = 64           # head dim
GH = 4            # heads per core
DG = GH * HD      # 256 projected cols per core
KD = D // P       # 8 k-tiles over model dim
NT = N // P       # 16 tiles over sequence
QC = 512          # n_q chunk size
NQC = N // QC     # 4 chunks
SCALE = HD ** -0.5

F32 = mybir.dt.float32
BF16 = mybir.dt.bfloat16

Exp = mybir.ActivationFunctionType.Exp


def build_nc():
    nc = bacc.Bacc("TRN2")

    xt = nc.declare_dram_parameter("xt", [D, N], BF16, isOutput=False)
    wq = nc.declare_dram_parameter("wq", [D, DG], BF16, isOutput=False)
    wk = nc.declare_dram_parameter("wk", [D, DG], BF16, isOutput=False)
    wv = nc.declare_dram_parameter("wv", [D, DG], BF16, isOutput=False)
    wo = nc.declare_dram_parameter("wo", [DG, D], BF16, isOutput=False)
    # bf16 output halves the 8MB/core y writeback (the kernel tail was
    # DMA-drain bound); host accumulates partials in fp32
    y = nc.declare_dram_parameter("y", [N, D], BF16, isOutput=True)

    xt_r = xt[:, :].rearrange("(o p) n -> p o n", p=P)    # [128, 8, 2048]
    wq_r = wq[:, :].rearrange("(o p) n -> p o n", p=P)    # [128, 8, 256]
    wk_r = wk[:, :].rearrange("(o p) n -> p o n", p=P)
    wv_r = wv[:, :].rearrange("(o p) n -> p o n", p=P)
    wo_r = wo[:, :].rearrange("(o p) n -> p o n", p=P)    # [128, 2, 1024]
    y_r = y[:, :].rearrange("(o p) n -> p o n", p=P)      # [128, 16, 1024]

    with tile.TileContext(nc) as tc, nc.allow_low_precision("bf16 attention"):
        with (
            tc.tile_pool(name="wpool", bufs=1) as wpool,
            tc.tile_pool(name="qkvpool", bufs=1) as qkvpool,
            tc.tile_pool(name="attnpool", bufs=1) as attnpool,
            tc.tile_pool(name="work", bufs=6) as work,
            tc.tile_pool(name="outp", bufs=2) as outp,
            # PSUM budget (8 banks): st 2x2 + o 3x1 + a 1x1 = 8.  Three o
            # slots let block B's first PV matmuls land in a fresh bank
            # instead of WAR-waiting on block A's epilogue multiplies.
            tc.tile_pool(name="ps_a", bufs=1, space="PSUM") as ps_a,
            tc.tile_pool(name="ps_st", bufs=2, space="PSUM") as ps_st,
            tc.tile_pool(name="ps_o", bufs=3, space="PSUM") as ps_o,
        ):
            # --- load weights (wk first: first matmuls need wk + xt k0) ---
            wk_sb = wpool.tile([P, KD, DG], BF16, tag="wk")
            nc.sync.dma_start(wk_sb[:], wk_r)
            wq_sb = wpool.tile([P, KD, DG], BF16, tag="wq")
            wv_sb = wpool.tile([P, KD, DG], BF16, tag="wv")
            wo_sb = wpool.tile([P, 2, D], BF16, tag="wo")

            # --- persistent tensors ---
            qt_sb = qkvpool.tile([P, 2, N], BF16, tag="qt")   # [256, 2048] qT
            kt_sb = qkvpool.tile([P, 2, N], BF16, tag="kt")   # [256, 2048] kT
            vg_sb = qkvpool.tile([P, NT, GH, 66], BF16, tag="vg")  # v + ones col
            nc.scalar.copy(
                vg_sb[:, :, :, HD:], nc.const_aps.tensor(1.0, (P, NT, GH, 2), F32)
            )
            at_sb = attnpool.tile([P, 2, N], BF16, tag="at")  # attn_outT [256, 2048]

            def alloc_o(c, pr):
                o_ps = []
                for half in range(2):
                    o_full = ps_o.tile(
                        [P, QC], F32, tag="o", name=f"o_{c}_{pr}_{half}"
                    )
                    o_ps.append(o_full[: HD + 1])
                return o_ps

            def emit_scores_act(c, pr, t):
                cs = slice(c * QC, (c + 1) * QC)
                ts_ = slice(t * P, (t + 1) * P)
                # both heads' transposed scores in one 2-bank tile
                st = ps_st.tile([P, 2, QC], F32, tag="st", name=f"st_{c}_{pr}_{t}")
                for half in range(2):
                    hs = slice(half * HD, (half + 1) * HD)
                    nc.tensor.matmul(
                        st[:, half, :],
                        kt_sb[hs, pr, ts_],
                        qt_sb[hs, pr, cs],
                        start=True,
                        stop=True,
                    )
                e = work.tile([P, 2, QC], BF16, tag="exp", name=f"e_{c}_{pr}_{t}")
                nc.scalar.activation(e[:], st[:], Exp, scale=SCALE)
                return e

            def emit_pv(c, pr, t, e, o_ps):
                last = None
                for half in range(2):
                    h = 2 * pr + half
                    last = nc.tensor.matmul(
                        o_ps[half][:],
                        vg_sb[:, t, h, 0:HD + 1],
                        e[:, half, :],
                        start=(t == 0),
                        stop=(t == NT - 1),
                    )
                return last

            def emit_epilogue(c, pr, o_ps, split=1, after_split=None):
                # normalize A^T = outT[:64] * (1/outT[64]): copy the denom
                # row to partition 0 (cross-quadrant row copy, probed legal),
                # reciprocal it with the fast custom-DVE op (requires base-0
                # APs), gpsimd-broadcast, then multiply STRAIGHT from the
                # PSUM accumulator into at_sb — half 1 writes partitions
                # 64..127 directly (out base 64, ins base 0: probed legal),
                # so no staging tile and no SBUF-SBUF DMA.
                # after_split: optional callback(s) run after split s is
                # written to at_sb (used to interleave tail proj units).
                w = QC // split
                for s in range(split):
                    ss = slice(s * w, (s + 1) * w)
                    cols = slice(c * QC + s * w, c * QC + (s + 1) * w)
                    for half in range(2):
                        sfx = f"{c}_{pr}_{half}_{s}"
                        dn = work.tile([1, w], F32, tag="dn", name=f"dn_{sfx}")
                        nc.vector.tensor_copy(dn[:], o_ps[half][HD:HD + 1, ss])
                        rc = work.tile([1, w], F32, tag="rc", name=f"rc_{sfx}")
                        nc.vector.reciprocal_approx_fast(rc[:], dn[:])
                        rbs = work.tile([HD, w], F32, tag="rbs", name=f"rbs_{sfx}")
                        nc.gpsimd.partition_broadcast(rbs[:], rc[:])
                        dst = (
                            at_sb[0:HD, pr, cols]
                            if half == 0
                            else at_sb[HD:P, pr, cols]
                        )
                        nc.vector.tensor_mul(dst, o_ps[half][0:HD, ss], rbs[:])
                    if after_split is not None:
                        after_split(s)

            def emit_proj_unit(unit, after=None):
                # one (m-tile, n-half) projection unit: 2 matmuls + copy + DMA
                m, nn = unit
                ps = ps_a.tile([P, QC], F32, tag="a", name=f"yps_{m}_{nn}")
                for ks in range(2):
                    mm = nc.tensor.matmul(
                        ps[:],
                        at_sb[:, ks, m * P:(m + 1) * P],
                        wo_sb[:, ks, nn * QC:(nn + 1) * QC],
                        start=(ks == 0),
                        stop=(ks == 1),
                    )
                    if after is not None and ks == 0:
                        # pin behind the gating attention matmul so the
                        # static scheduler doesn't hoist the projection
                        # ahead of the (slow) normalization chain
                        bass._add_dep_helper(
                            mm.ins, after.ins, sync=False, reason="defer proj"
                        )
                ysb = outp.tile([P, QC], BF16, tag="y", name=f"y_{m}_{nn}")
                nc.vector.tensor_copy(ysb[:], ps[:])
                nc.sync.dma_start(y_r[:, m, nn * QC:(nn + 1) * QC], ysb[:])

            # ------------- qkv projections + interleaved attention -----------
            with tc.tile_pool(name="xpool", bufs=1) as xpool:
                # load xt in COLUMN chunks (all 8 k-planes per 512-col range):
                # kt/qt chunk c and v tiles t=4c..4c+3 consume only columns
                # [512c, 512c+512), so the first attention work starts after
                # 1MB instead of the full 4MB transfer
                xt_sb = xpool.tile([P, KD, N], BF16, tag="xt")
                nc.sync.dma_start(xt_sb[:, :, 0:QC], xt_r[:, :, 0:QC])
                nc.sync.dma_start(wq_sb[:], wq_r)
                nc.sync.dma_start(xt_sb[:, :, QC:2 * QC], xt_r[:, :, QC:2 * QC])
                nc.sync.dma_start(wv_sb[:], wv_r)
                nc.sync.dma_start(
                    xt_sb[:, :, 2 * QC:3 * QC], xt_r[:, :, 2 * QC:3 * QC]
                )
                nc.sync.dma_start(wo_sb[:], wo_r)
                nc.sync.dma_start(
                    xt_sb[:, :, 3 * QC:4 * QC], xt_r[:, :, 3 * QC:4 * QC]
                )

                def emit_kq_group(which, w_sb, dst, m, c):
                    ps = ps_a.tile([P, QC], F32, tag="a", name=f"{which}ps_{m}_{c}")
                    for k in range(KD):
                        nc.tensor.matmul(
                            ps[:],
                            w_sb[:, k, m * P:(m + 1) * P],
                            xt_sb[:, k, c * QC:(c + 1) * QC],
                            start=(k == 0),
                            stop=(k == KD - 1),
                        )
                    nc.vector.tensor_copy(dst[:, m, c * QC:(c + 1) * QC], ps[:])

                # minimal prefix: kT m0c0 + qT m0c0 — everything else is
                # emitted just-in-time inside the attention stream so ScalarE
                # (the bottleneck engine) saturates as early as possible
                emit_kq_group("k", wk_sb, kt_sb, 0, 0)
                emit_kq_group("q", wq_sb, qt_sb, 0, 0)

                def emit_v(t):
                    # v = x @ wv -> vg_sb[t] per-head (interleaved in block 0)
                    ps = ps_a.tile([P, QC], F32, tag="a", name=f"vps_{t}")
                    for k in range(KD):
                        nc.tensor.matmul(
                            ps[:, :DG],
                            xt_sb[:, k, t * P:(t + 1) * P],
                            wv_sb[:, k, :],
                            start=(k == 0),
                            stop=(k == KD - 1),
                        )
                    nc.vector.tensor_copy(
                        vg_sb[:, t, :, 0:HD],
                        ps[:, :DG].rearrange("p (h e) -> p h e", h=GH),
                    )

                # per-block {t: group} JIT emissions: block (0,0) computes the
                # kt groups it consumes itself plus kt m1 c0 / qt m1 c0 for
                # (0,1); (0,1) computes its own later kt m1 chunks; each block
                # emits the q chunk needed two blocks ahead
                kq_jit = {
                    (0, 0): {
                        0: ("k", wk_sb, kt_sb, 0, 1),
                        1: ("k", wk_sb, kt_sb, 0, 2),
                        2: ("k", wk_sb, kt_sb, 0, 3),
                        5: ("k", wk_sb, kt_sb, 1, 0),
                        13: ("q", wq_sb, qt_sb, 1, 0),
                    },
                    (0, 1): {
                        0: ("k", wk_sb, kt_sb, 1, 1),
                        3: ("q", wq_sb, qt_sb, 0, 1),
                        4: ("k", wk_sb, kt_sb, 1, 2),
                        9: ("k", wk_sb, kt_sb, 1, 3),
                    },
                    (1, 0): {3: ("q", wq_sb, qt_sb, 1, 1)},
                    (1, 1): {3: ("q", wq_sb, qt_sb, 0, 2)},
                    (2, 0): {3: ("q", wq_sb, qt_sb, 1, 2)},
                    (2, 1): {3: ("q", wq_sb, qt_sb, 0, 3)},
                    (3, 0): {3: ("q", wq_sb, qt_sb, 1, 3)},
                }

                # ---- flat stream over all (block, t) steps with the PV
                # matmuls software-pipelined THREE iterations behind their
                # scores/exp: by the time PV(t) is issued, ACT(t) has long
                # finished, so the in-order PE queue never stalls on ScalarE
                # and the LDWEIGHTS prefetch stays hidden.  At each block
                # start the previous block's remaining PVs flush as a burst
                # (covered by ACT(B,0)) so its epilogue chain starts early.
                LAG = 3
                pending_proj = []
                pv_q = []
                epi_q = None
                last_gate = [None]

                def pop_pv():
                    last_gate[0] = emit_pv(*pv_q.pop(0))

                for c, pr in [(0, 0), (0, 1), (1, 0), (1, 1),
                              (2, 0), (2, 1), (3, 0), (3, 1)]:
                    o_ps = alloc_o(c, pr)
                    jit = kq_jit.get((c, pr), {})
                    for t in range(NT):
                        if (c, pr) == (0, 0):
                            emit_v(t)
                        e = emit_scores_act(c, pr, t)
                        if len(pv_q) >= LAG:
                            pop_pv()
                        pv_q.append((c, pr, t, e, o_ps))
                        if t == 0 and epi_q is not None:
                            while len(pv_q) > 1:
                                pop_pv()
                            emit_epilogue(*epi_q, split=2)
                            epi_q = None
                        if t in jit:
                            emit_kq_group(*jit[t])
                        if pending_proj and t in (5, 7, 9, 11, 13):
                            emit_proj_unit(pending_proj.pop(0), after=last_gate[0])
                    epi_q = (c, pr, o_ps)
                    if pr == 1:
                        pending_proj.extend(
                            (4 * c + mi, nn) for mi in range(4) for nn in range(2)
                        )

                # drain: remaining PVs, final epilogue with chunk-3 proj
                # units interleaved between its splits (split s covers
                # m-tiles 12+2s..13+2s)
                while pv_q:
                    pop_pv()

                def tail_proj(s):
                    for mi in (12 + 2 * s, 13 + 2 * s):
                        for nn in range(2):
                            emit_proj_unit((mi, nn))

                emit_epilogue(*epi_q, split=2, after_split=tail_proj)
                for unit in pending_proj:
                    if unit[0] < 12:
                        emit_proj_unit(unit)

    nc.finalize()
    return nc


_NC = None


def _get_nc():
    global _NC
    if _NC is None:
        _NC = build_nc()
    return _NC


def _in_maps(x, w_qkv, w_out):
    bf = ml_dtypes.bfloat16
    x = np.asarray(x, dtype=np.float32)
    w_qkv = np.asarray(w_qkv, dtype=np.float32)
    w_out = np.asarray(w_out, dtype=np.float32)
    xts = [np.ascontiguousarray(x[b].T).astype(bf) for b in range(2)]
    wq_g = [np.ascontiguousarray(w_qkv[:, 0 * D + g * DG:0 * D + (g + 1) * DG]).astype(bf) for g in range(4)]
    wk_g = [np.ascontiguousarray(w_qkv[:, 1 * D + g * DG:1 * D + (g + 1) * DG]).astype(bf) for g in range(4)]
    wv_g = [np.ascontiguousarray(w_qkv[:, 2 * D + g * DG:2 * D + (g + 1) * DG]).astype(bf) for g in range(4)]
    wo_g = [np.ascontiguousarray(w_out[g * DG:(g + 1) * DG, :]).astype(bf) for g in range(4)]
    maps = []
    for c in range(8):
        b, g = c // 4, c % 4
        maps.append({
            "xt": xts[b],
            "wq": wq_g[g],
            "wk": wk_g[g],
            "wv": wv_g[g],
            "wo": wo_g[g],
        })
    return maps


LAST_RESULT = None


def kernel(x, w_qkv, w_out, b_out):
    from concourse.bass_utils import run_bass_kernel_spmd

    nc = _get_nc()
    maps = _in_maps(x, w_qkv, w_out)
    res = run_bass_kernel_spmd(nc, maps, list(range(8)))
    global LAST_RESULT
    LAST_RESULT = res
    out = np.zeros((2, N, D), dtype=np.float32)
    for c in range(8):
        out[c // 4] += np.asarray(res.results[c]["y"], dtype=np.float32)
    out += np.asarray(b_out, dtype=np.float32)[None, None, :]
    return out



# revision 18
# speedup vs baseline: 1.1295x; 1.1295x over previous
"""Multi-head attention (b=2, n=2048, d=1024, H=16 heads) on 8 TRN2 NeuronCores.

Sharding: core c = (b, g) with b = c // 4 (data parallel over batch) and
g = c % 4 (tensor parallel over head groups of 4 heads).  Each core computes
qkv projections for its 4 heads, full softmax attention for those heads, and
a partial output projection y_partial = A_heads @ w_out[g*256:(g+1)*256].
The host sums the 4 partials per batch and adds b_out.

Layout strategy (per core):
  - host passes xT = x[b].T  [1024, 2048] in bf16 (d on partitions when tiled)
  - qT, kT computed as [256, 2048] (head_dim*heads on partitions) via
    matmul(lhsT=w_slice, rhs=xT); head pairs (2m, 2m+1) sit on partition
    halves of m-tile m so the two K=64 score matmuls run concurrently in
    disjoint PE row groups
  - v computed as [2048, 256] via matmul(lhsT=xT, rhs=wv), stored per-head
    with an appended ones column (v_aug [nk, 65]) so the PV matmul also
    accumulates the softmax denominator
  - scores computed TRANSPOSED: ST[nk, nq] = matmul(lhsT=kT, rhs=qT); the two
    heads of a pair share one 2-bank PSUM tile so a single ACTIVATE exps 1024
    elements (ScalarE is the second-busiest engine; its per-op overhead is
    ~352 cycles)
  - softmax needs no row-max subtraction (scores ~ N(0,1), exp <= ~3e3)
  - PV: outT[65, nq] += matmul(lhsT=v_aug, rhs=exp(ST))
  - accumulators are staged to SBUF immediately (frees PSUM for the next
    pair) and normalized there: DVE reciprocal + gpsimd partition_broadcast
  - output projection for chunk c is emitted interleaved into the NEXT
    pair-block's inner loop so the PE never waits on the normalization chain
  - the first attention block is interleaved with the v projection so ScalarE
    starts exp work early
Matmuls run in bf16 (fp32 PSUM accumulation); measured end-to-end relative
error ~5e-3 vs the fp32 reference.
"""

import os
import sys

for _p in ("/opt/trn_rl_repo",):
    if _p not in sys.path and os.path.isdir(_p):
        sys.path.insert(0, _p)

import ml_dtypes
import numpy as np

import concourse.bass as bass
import concourse.mybir as mybir
import concourse.tile as tile
from concourse import bacc

P = 128
D = 1024          # model dim
N = 2048          # sequence length
HD = 64           # head dim
GH = 4            # heads per core
DG = GH * HD      # 256 projected cols per core
KD = D // P       # 8 k-tiles over model dim
NT = N // P       # 16 tiles over sequence
QC = 512          # n_q chunk size
NQC = N // QC     # 4 chunks
SCALE = HD ** -0.5

F32 = mybir.dt.float32
BF16 = mybir.dt.bfloat16

Exp = mybir.ActivationFunctionType.Exp


def build_nc():
    nc = bacc.Bacc("TRN2")

    xt = nc.declare_dram_parameter("xt", [D, N], BF16, isOutput=False)
    wq = nc.declare_dram_parameter("wq", [D, DG], BF16, isOutput=False)
    wk = nc.declare_dram_parameter("wk", [D, DG], BF16, isOutput=False)
    wv = nc.declare_dram_parameter("wv", [D, DG], BF16, isOutput=False)
    wo = nc.declare_dram_parameter("wo", [DG, D], BF16, isOutput=False)
    # bf16 output halves the 8MB/core y writeback (the kernel tail was
    # DMA-drain bound); host accumulates partials in fp32
    y = nc.declare_dram_parameter("y", [N, D], BF16, isOutput=True)

    xt_r = xt[:, :].rearrange("(o p) n -> p o n", p=P)    # [128, 8, 2048]
    wq_r = wq[:, :].rearrange("(o p) n -> p o n", p=P)    # [128, 8, 256]
    wk_r = wk[:, :].rearrange("(o p) n -> p o n", p=P)
    wv_r = wv[:, :].rearrange("(o p) n -> p o n", p=P)
    wo_r = wo[:, :].rearrange("(o p) n -> p o n", p=P)    # [128, 2, 1024]
    y_r = y[:, :].rearrange("(o p) n -> p o n", p=P)      # [128, 16, 1024]

    with tile.TileContext(nc) as tc, nc.allow_low_precision("bf16 attention"):
        with (
            tc.tile_pool(name="wpool", bufs=1) as wpool,
            tc.tile_pool(name="qkvpool", bufs=1) as qkvpool,
            tc.tile_pool(name="attnpool", bufs=1) as attnpool,
            tc.tile_pool(name="work", bufs=6) as work,
            tc.tile_pool(name="outp", bufs=2) as outp,
            # PSUM budget (8 banks): st 2x2 + o 2x1 + a 2x1 = 8.  The
            # boundary flush (all prior-block PVs popped at B's t=0) makes
            # A's epilogue multiplies finish before B's PV t=0 needs A's
            # accumulator banks, so o double-buffering suffices.
            tc.tile_pool(name="ps_a", bufs=2, space="PSUM") as ps_a,
            tc.tile_pool(name="ps_st", bufs=2, space="PSUM") as ps_st,
            tc.tile_pool(name="ps_o", bufs=2, space="PSUM") as ps_o,
        ):
            # --- load weights (wk first: first matmuls need wk + xt k0) ---
            wk_sb = wpool.tile([P, KD, DG], BF16, tag="wk")
            nc.sync.dma_start(wk_sb[:], wk_r)
            wq_sb = wpool.tile([P, KD, DG], BF16, tag="wq")
            wv_sb = wpool.tile([P, KD, DG], BF16, tag="wv")
            wo_sb = wpool.tile([P, 2, D], BF16, tag="wo")

            # --- persistent tensors ---
            qt_sb = qkvpool.tile([P, 2, N], BF16, tag="qt")   # [256, 2048] qT
            kt_sb = qkvpool.tile([P, 2, N], BF16, tag="kt")   # [256, 2048] kT
            vg_sb = qkvpool.tile([P, NT, GH, 66], BF16, tag="vg")  # v + ones col
            nc.scalar.copy(
                vg_sb[:, :, :, HD:], nc.const_aps.tensor(1.0, (P, NT, GH, 2), F32)
            )
            at_sb = attnpool.tile([P, 2, N], BF16, tag="at")  # attn_outT [256, 2048]

            def alloc_o(c, pr):
                o_ps = []
                for half in range(2):
                    o_full = ps_o.tile(
                        [P, QC], F32, tag="o", name=f"o_{c}_{pr}_{half}"
                    )
                    o_ps.append(o_full[: HD + 1])
                return o_ps

            def emit_scores_act(c, pr, t):
                cs = slice(c * QC, (c + 1) * QC)
                ts_ = slice(t * P, (t + 1) * P)
                # both heads' transposed scores in one 2-bank tile
                st = ps_st.tile([P, 2, QC], F32, tag="st", name=f"st_{c}_{pr}_{t}")
                for half in range(2):
                    hs = slice(half * HD, (half + 1) * HD)
                    nc.tensor.matmul(
                        st[:, half, :],
                        kt_sb[hs, pr, ts_],
                        qt_sb[hs, pr, cs],
                        start=True,
                        stop=True,
                    )
                e = work.tile([P, 2, QC], BF16, tag="exp", name=f"e_{c}_{pr}_{t}")
                nc.scalar.activation(e[:], st[:], Exp, scale=SCALE)
                return e

            def emit_pv(c, pr, t, e, o_ps):
                last = None
                for half in range(2):
                    h = 2 * pr + half
                    last = nc.tensor.matmul(
                        o_ps[half][:],
                        vg_sb[:, t, h, 0:HD + 1],
                        e[:, half, :],
                        start=(t == 0),
                        stop=(t == NT - 1),
                    )
                return last

            def emit_epilogue(c, pr, o_ps, split=1, after_split=None):
                # normalize A^T = outT[:64] * (1/outT[64]): copy the denom
                # row to partition 0 (cross-quadrant row copy, probed legal),
                # reciprocal it with the fast custom-DVE op (requires base-0
                # APs), gpsimd-broadcast, then multiply STRAIGHT from the
                # PSUM accumulator into at_sb — half 1 writes partitions
                # 64..127 directly (out base 64, ins base 0: probed legal),
                # so no staging tile and no SBUF-SBUF DMA.
                # after_split: optional callback(s) run after split s is
                # written to at_sb (used to interleave tail proj units).
                w = QC // split
                for s in range(split):
                    ss = slice(s * w, (s + 1) * w)
                    cols = slice(c * QC + s * w, c * QC + (s + 1) * w)
                    for half in range(2):
                        sfx = f"{c}_{pr}_{half}_{s}"
                        dn = work.tile([1, w], F32, tag="dn", name=f"dn_{sfx}")
                        nc.vector.tensor_copy(dn[:], o_ps[half][HD:HD + 1, ss])
                        rc = work.tile([1, w], F32, tag="rc", name=f"rc_{sfx}")
                        nc.vector.reciprocal_approx_fast(rc[:], dn[:])
                        rbs = work.tile([HD, w], F32, tag="rbs", name=f"rbs_{sfx}")
                        nc.gpsimd.partition_broadcast(rbs[:], rc[:])
                        dst = (
                            at_sb[0:HD, pr, cols]
                            if half == 0
                            else at_sb[HD:P, pr, cols]
                        )
                        nc.vector.tensor_mul(dst, o_ps[half][0:HD, ss], rbs[:])
                    if after_split is not None:
                        after_split(s)

            def emit_proj_unit(unit, after=None):
                # one (m-tile, n-half) projection unit: 2 matmuls + copy + DMA
                m, nn = unit
                ps = ps_a.tile([P, QC], F32, tag="a", name=f"yps_{m}_{nn}")
                for ks in range(2):
                    mm = nc.tensor.matmul(
                        ps[:],
                        at_sb[:, ks, m * P:(m + 1) * P],
                        wo_sb[:, ks, nn * QC:(nn + 1) * QC],
                        start=(ks == 0),
                        stop=(ks == 1),
                    )
                    if after is not None and ks == 0:
                        # pin behind the gating attention matmul so the
                        # static scheduler doesn't hoist the projection
                        # ahead of the (slow) normalization chain
                        bass._add_dep_helper(
                            mm.ins, after.ins, sync=False, reason="defer proj"
                        )
                ysb = outp.tile([P, QC], BF16, tag="y", name=f"y_{m}_{nn}")
                nc.vector.tensor_copy(ysb[:], ps[:])
                nc.sync.dma_start(y_r[:, m, nn * QC:(nn + 1) * QC], ysb[:])

            # ------------- qkv projections + interleaved attention -----------
            with tc.tile_pool(name="xpool", bufs=1) as xpool:
                # load xt in COLUMN chunks (all 8 k-planes per 512-col range):
                # kt/qt chunk c and v tiles t=4c..4c+3 consume only columns
                # [512c, 512c+512), so the first attention work starts after
                # 1MB instead of the full 4MB transfer
                xt_sb = xpool.tile([P, KD, N], BF16, tag="xt")
                nc.sync.dma_start(xt_sb[:, :, 0:QC], xt_r[:, :, 0:QC])
                nc.sync.dma_start(wq_sb[:], wq_r)
                nc.sync.dma_start(xt_sb[:, :, QC:2 * QC], xt_r[:, :, QC:2 * QC])
                nc.sync.dma_start(wv_sb[:], wv_r)
                nc.sync.dma_start(
                    xt_sb[:, :, 2 * QC:3 * QC], xt_r[:, :, 2 * QC:3 * QC]
                )
                nc.sync.dma_start(wo_sb[:], wo_r)
                nc.sync.dma_start(
                    xt_sb[:, :, 3 * QC:4 * QC], xt_r[:, :, 3 * QC:4 * QC]
                )

                def emit_kq_group(which, w_sb, dst, m, c):
                    ps = ps_a.tile([P, QC], F32, tag="a", name=f"{which}ps_{m}_{c}")
                    for k in range(KD):
                        nc.tensor.matmul(
                            ps[:],
                            w_sb[:, k, m * P:(m + 1) * P],
                            xt_sb[:, k, c * QC:(c + 1) * QC],
                            start=(k == 0),
                            stop=(k == KD - 1),
                        )
                    nc.vector.tensor_copy(dst[:, m, c * QC:(c + 1) * QC], ps[:])

                # minimal prefix: kT m0c0 + qT m0c0 — everything else is
                # emitted just-in-time inside the attention stream so ScalarE
                # (the bottleneck engine) saturates as early as possible
                emit_kq_group("k", wk_sb, kt_sb, 0, 0)
                emit_kq_group("q", wq_sb, qt_sb, 0, 0)

                def emit_v(t):
                    # v = x @ wv -> vg_sb[t] per-head (interleaved in block 0)
                    ps = ps_a.tile([P, QC], F32, tag="a", name=f"vps_{t}")
                    for k in range(KD):
                        nc.tensor.matmul(
                            ps[:, :DG],
                            xt_sb[:, k, t * P:(t + 1) * P],
                            wv_sb[:, k, :],
                            start=(k == 0),
                            stop=(k == KD - 1),
                        )
                    nc.vector.tensor_copy(
                        vg_sb[:, t, :, 0:HD],
                        ps[:, :DG].rearrange("p (h e) -> p h e", h=GH),
                    )

                # per-block {t: group} JIT emissions: block (0,0) computes the
                # kt groups it consumes itself plus kt m1 c0 / qt m1 c0 for
                # (0,1); (0,1) computes its own later kt m1 chunks; each block
                # emits the q chunk needed two blocks ahead
                kq_jit = {
                    (0, 0): {
                        0: ("k", wk_sb, kt_sb, 0, 1),
                        1: ("k", wk_sb, kt_sb, 0, 2),
                        2: ("k", wk_sb, kt_sb, 0, 3),
                        5: ("k", wk_sb, kt_sb, 1, 0),
                        13: ("q", wq_sb, qt_sb, 1, 0),
                    },
                    (0, 1): {
                        0: ("k", wk_sb, kt_sb, 1, 1),
                        3: ("q", wq_sb, qt_sb, 0, 1),
                        4: ("k", wk_sb, kt_sb, 1, 2),
                        9: ("k", wk_sb, kt_sb, 1, 3),
                    },
                    (1, 0): {3: ("q", wq_sb, qt_sb, 1, 1)},
                    (1, 1): {3: ("q", wq_sb, qt_sb, 0, 2)},
                    (2, 0): {3: ("q", wq_sb, qt_sb, 1, 2)},
                    (2, 1): {3: ("q", wq_sb, qt_sb, 0, 3)},
                    (3, 0): {3: ("q", wq_sb, qt_sb, 1, 3)},
                }

                # ---- flat stream over all (block, t) steps with the PV
                # matmuls software-pipelined THREE iterations behind their
                # scores/exp: by the time PV(t) is issued, ACT(t) has long
                # finished, so the in-order PE queue never stalls on ScalarE
                # and the LDWEIGHTS prefetch stays hidden.  At each block
                # start the previous block's remaining PVs flush as a burst
                # (covered by ACT(B,0)) so its epilogue chain starts early.
                LAG = 3
                pending_proj = []
                pv_q = []
                epi_q = None
                last_gate = [None]

                def pop_pv():
                    last_gate[0] = emit_pv(*pv_q.pop(0))

                for c, pr in [(0, 0), (0, 1), (1, 0), (1, 1),
                              (2, 0), (2, 1), (3, 0), (3, 1)]:
                    o_ps = alloc_o(c, pr)
                    jit = kq_jit.get((c, pr), {})
                    for t in range(NT):
                        if (c, pr) == (0, 0):
                            emit_v(t)
                        e = emit_scores_act(c, pr, t)
                        if len(pv_q) >= LAG:
                            pop_pv()
                        pv_q.append((c, pr, t, e, o_ps))
                        if t == 0 and epi_q is not None:
                            while len(pv_q) > 1:
                                pop_pv()
                            emit_epilogue(*epi_q, split=2)
                            epi_q = None
                        if t in jit:
                            emit_kq_group(*jit[t])
                        # (3,1) keeps 2 chunk-2 units back to fill the PE
                        # during the drain epilogue chain (avoids the >3.4us
                        # PE idle that drops the HAM clock to 1.2GHz)
                        slots = (5,) if (c, pr) == (3, 1) else (5, 7, 9, 11, 13)
                        if pending_proj and t in slots:
                            emit_proj_unit(pending_proj.pop(0), after=last_gate[0])
                    epi_q = (c, pr, o_ps)
                    if pr == 1:
                        pending_proj.extend(
                            (4 * c + mi, nn) for mi in range(4) for nn in range(2)
                        )

                # drain: remaining PVs, held-back chunk-2 units (PE filler
                # under the final epilogue chain), then the final epilogue
                # with chunk-3 proj units interleaved between its splits
                # (split s covers m-tiles 12+2s..13+2s)
                while pv_q:
                    pop_pv()
                for unit in pending_proj:
                    emit_proj_unit(unit)

                def tail_proj(s):
                    for mi in (12 + 2 * s, 13 + 2 * s):
                        for nn in range(2):
                            emit_proj_unit((mi, nn))

                emit_epilogue(*epi_q, split=2, after_split=tail_proj)

    nc.finalize()
    return nc


_NC = None


def _get_nc():
    global _NC
    if _NC is None:
        _NC = build_nc()
    return _NC


def _in_maps(x, w_qkv, w_out):
    bf = ml_dtypes.bfloat16
    x = np.asarray(x, dtype=np.float32)
    w_qkv = np.asarray(w_qkv, dtype=np.float32)
    w_out = np.asarray(w_out, dtype=np.float32)
    xts = [np.ascontiguousarray(x[b].T).astype(bf) for b in range(2)]
    wq_g = [np.ascontiguousarray(w_qkv[:, 0 * D + g * DG:0 * D + (g + 1) * DG]).astype(bf) for g in range(4)]
    wk_g = [np.ascontiguousarray(w_qkv[:, 1 * D + g * DG:1 * D + (g + 1) * DG]).astype(bf) for g in range(4)]
    wv_g = [np.ascontiguousarray(w_qkv[:, 2 * D + g * DG:2 * D + (g + 1) * DG]).astype(bf) for g in range(4)]
    wo_g = [np.ascontiguousarray(w_out[g * DG:(g + 1) * DG, :]).astype(bf) for g in range(4)]
    maps = []
    for c in range(8):
        b, g = c // 4, c % 4
        maps.append({
            "xt": xts[b],
            "wq": wq_g[g],
            "wk": wk_g[g],
            "wv": wv_g[g],
            "wo": wo_g[g],
        })
    return maps


LAST_RESULT = None


def kernel(x, w_qkv, w_out, b_out):
    from concourse.bass_utils import run_bass_kernel_spmd

    nc = _get_nc()
    maps = _in_maps(x, w_qkv, w_out)
    res = run_bass_kernel_spmd(nc, maps, list(range(8)))
    global LAST_RESULT
    LAST_RESULT = res
    out = np.zeros((2, N, D), dtype=np.float32)
    for c in range(8):
        out[c // 4] += np.asarray(res.results[c]["y"], dtype=np.float32)
    out += np.asarray(b_out, dtype=np.float32)[None, None, :]
    return out



# revision 19
# speedup vs baseline: 1.1799x; 1.0446x over previous
"""Multi-head attention (b=2, n=2048, d=1024, H=16 heads) on 8 TRN2 NeuronCores.

Sharding: core c = (b, g) with b = c // 4 (data parallel over batch) and
g = c % 4 (tensor parallel over head groups of 4 heads).  Each core computes
qkv projections for its 4 heads, full softmax attention for those heads, and
a partial output projection y_partial = A_heads @ w_out[g*256:(g+1)*256].
The host sums the 4 partials per batch and adds b_out.

Layout strategy (per core):
  - host passes xT = x[b].T  [1024, 2048] in bf16 (d on partitions when tiled)
  - qT, kT computed as [256, 2048] (head_dim*heads on partitions) via
    matmul(lhsT=w_slice, rhs=xT); head pairs (2m, 2m+1) sit on partition
    halves of m-tile m so the two K=64 score matmuls run concurrently in
    disjoint PE row groups
  - v computed as [2048, 256] via matmul(lhsT=xT, rhs=wv), stored per-head
    with an appended ones column (v_aug [nk, 65]) so the PV matmul also
    accumulates the softmax denominator
  - scores computed TRANSPOSED: ST[nk, nq] = matmul(lhsT=kT, rhs=qT); the two
    heads of a pair share one 2-bank PSUM tile so a single ACTIVATE exps 1024
    elements (ScalarE is the second-busiest engine; its per-op overhead is
    ~352 cycles)
  - softmax needs no row-max subtraction (scores ~ N(0,1), exp <= ~3e3)
  - PV: outT[65, nq] += matmul(lhsT=v_aug, rhs=exp(ST))
  - accumulators are staged to SBUF immediately (frees PSUM for the next
    pair) and normalized there: DVE reciprocal + gpsimd partition_broadcast
  - output projection for chunk c is emitted interleaved into the NEXT
    pair-block's inner loop so the PE never waits on the normalization chain
  - the first attention block is interleaved with the v projection so ScalarE
    starts exp work early
Matmuls run in bf16 (fp32 PSUM accumulation); measured end-to-end relative
error ~5e-3 vs the fp32 reference.
"""

import os
import sys

for _p in ("/opt/trn_rl_repo",):
    if _p not in sys.path and os.path.isdir(_p):
        sys.path.insert(0, _p)

import ml_dtypes
import numpy as np

import concourse.bass as bass
import concourse.mybir as mybir
import concourse.tile as tile
from concourse import bacc

P = 128
D = 1024          # model dim
N = 2048          # sequence length
HD = 64           # head dim
GH = 4            # heads per core
DG = GH * HD      # 256 projected cols per core
KD = D // P       # 8 k-tiles over model dim
NT = N // P       # 16 tiles over sequence
QC = 512          # n_q chunk size
NQC = N // QC     # 4 chunks
SCALE = HD ** -0.5

F32 = mybir.dt.float32
BF16 = mybir.dt.bfloat16

Exp = mybir.ActivationFunctionType.Exp


def build_nc():
    nc = bacc.Bacc("TRN2")

    xt = nc.declare_dram_parameter("xt", [D, N], BF16, isOutput=False)
    wq = nc.declare_dram_parameter("wq", [D, DG], BF16, isOutput=False)
    wk = nc.declare_dram_parameter("wk", [D, DG], BF16, isOutput=False)
    wv = nc.declare_dram_parameter("wv", [D, DG], BF16, isOutput=False)
    wo = nc.declare_dram_parameter("wo", [DG, D], BF16, isOutput=False)
    # bf16 output halves the 8MB/core y writeback (the kernel tail was
    # DMA-drain bound); host accumulates partials in fp32
    y = nc.declare_dram_parameter("y", [N, D], BF16, isOutput=True)

    xt_r = xt[:, :].rearrange("(o p) n -> p o n", p=P)    # [128, 8, 2048]
    wq_r = wq[:, :].rearrange("(o p) n -> p o n", p=P)    # [128, 8, 256]
    wk_r = wk[:, :].rearrange("(o p) n -> p o n", p=P)
    wv_r = wv[:, :].rearrange("(o p) n -> p o n", p=P)
    wo_r = wo[:, :].rearrange("(o p) n -> p o n", p=P)    # [128, 2, 1024]
    y_r = y[:, :].rearrange("(o p) n -> p o n", p=P)      # [128, 16, 1024]

    with tile.TileContext(nc) as tc, nc.allow_low_precision("bf16 attention"):
        with (
            tc.tile_pool(name="wpool", bufs=1) as wpool,
            tc.tile_pool(name="qkvpool", bufs=1) as qkvpool,
            tc.tile_pool(name="attnpool", bufs=1) as attnpool,
            tc.tile_pool(name="work", bufs=6) as work,
            tc.tile_pool(name="outp", bufs=2) as outp,
            # PSUM budget (8 banks): st 2x2 + o 2x1 + a 2x1 = 8.  The
            # boundary flush (all prior-block PVs popped at B's t=0) makes
            # A's epilogue multiplies finish before B's PV t=0 needs A's
            # accumulator banks, so o double-buffering suffices.
            tc.tile_pool(name="ps_a", bufs=2, space="PSUM") as ps_a,
            tc.tile_pool(name="ps_st", bufs=2, space="PSUM") as ps_st,
            tc.tile_pool(name="ps_o", bufs=2, space="PSUM") as ps_o,
        ):
            # --- load weights (wk first: first matmuls need wk + xt k0) ---
            wk_sb = wpool.tile([P, KD, DG], BF16, tag="wk")
            nc.sync.dma_start(wk_sb[:], wk_r)
            wq_sb = wpool.tile([P, KD, DG], BF16, tag="wq")
            wv_sb = wpool.tile([P, KD, DG], BF16, tag="wv")
            wo_sb = wpool.tile([P, 2, D], BF16, tag="wo")

            # --- persistent tensors ---
            qt_sb = qkvpool.tile([P, 2, N], BF16, tag="qt")   # [256, 2048] qT
            kt_sb = qkvpool.tile([P, 2, N], BF16, tag="kt")   # [256, 2048] kT
            vg_sb = qkvpool.tile([P, NT, GH, 66], BF16, tag="vg")  # v + ones col
            nc.scalar.copy(
                vg_sb[:, :, :, HD:], nc.const_aps.tensor(1.0, (P, NT, GH, 2), F32)
            )
            at_sb = attnpool.tile([P, 2, N], BF16, tag="at")  # attn_outT [256, 2048]

            def alloc_o(c, pr):
                o_ps = []
                for half in range(2):
                    o_full = ps_o.tile(
                        [P, QC], F32, tag="o", name=f"o_{c}_{pr}_{half}"
                    )
                    o_ps.append(o_full[: HD + 1])
                return o_ps

            def emit_scores_act(c, pr, t):
                cs = slice(c * QC, (c + 1) * QC)
                ts_ = slice(t * P, (t + 1) * P)
                # both heads' transposed scores in one 2-bank tile
                st = ps_st.tile([P, 2, QC], F32, tag="st", name=f"st_{c}_{pr}_{t}")
                for half in range(2):
                    hs = slice(half * HD, (half + 1) * HD)
                    nc.tensor.matmul(
                        st[:, half, :],
                        kt_sb[hs, pr, ts_],
                        qt_sb[hs, pr, cs],
                        start=True,
                        stop=True,
                    )
                e = work.tile([P, 2, QC], BF16, tag="exp", name=f"e_{c}_{pr}_{t}")
                nc.scalar.activation(e[:], st[:], Exp, scale=SCALE)
                return e

            def emit_pv(c, pr, t, e, o_ps):
                last = None
                for half in range(2):
                    h = 2 * pr + half
                    last = nc.tensor.matmul(
                        o_ps[half][:],
                        vg_sb[:, t, h, 0:HD + 1],
                        e[:, half, :],
                        start=(t == 0),
                        stop=(t == NT - 1),
                    )
                return last

            def emit_epilogue(c, pr, o_ps, split=1, after_split=None):
                # normalize A^T = outT[:64] * (1/outT[64]): copy the denom
                # row to partition 0 (cross-quadrant row copy, probed legal),
                # reciprocal it with the fast custom-DVE op (requires base-0
                # APs), gpsimd-broadcast, then multiply STRAIGHT from the
                # PSUM accumulator into at_sb — half 1 writes partitions
                # 64..127 directly (out base 64, ins base 0: probed legal),
                # so no staging tile and no SBUF-SBUF DMA.
                # after_split: optional callback(s) run after split s is
                # written to at_sb (used to interleave tail proj units).
                w = QC // split
                for s in range(split):
                    ss = slice(s * w, (s + 1) * w)
                    cols = slice(c * QC + s * w, c * QC + (s + 1) * w)
                    for half in range(2):
                        sfx = f"{c}_{pr}_{half}_{s}"
                        dn = work.tile([1, w], F32, tag="dn", name=f"dn_{sfx}")
                        nc.vector.tensor_copy(dn[:], o_ps[half][HD:HD + 1, ss])
                        rc = work.tile([1, w], F32, tag="rc", name=f"rc_{sfx}")
                        nc.vector.reciprocal_approx_fast(rc[:], dn[:])
                        rbs = work.tile([HD, w], F32, tag="rbs", name=f"rbs_{sfx}")
                        nc.gpsimd.partition_broadcast(rbs[:], rc[:])
                        dst = (
                            at_sb[0:HD, pr, cols]
                            if half == 0
                            else at_sb[HD:P, pr, cols]
                        )
                        nc.vector.tensor_mul(dst, o_ps[half][0:HD, ss], rbs[:])
                    if after_split is not None:
                        after_split(s)

            def emit_proj_unit(unit, after=None):
                # one (m-tile, n-half) projection unit: 2 matmuls + copy + DMA
                m, nn = unit
                ps = ps_a.tile([P, QC], F32, tag="a", name=f"yps_{m}_{nn}")
                for ks in range(2):
                    mm = nc.tensor.matmul(
                        ps[:],
                        at_sb[:, ks, m * P:(m + 1) * P],
                        wo_sb[:, ks, nn * QC:(nn + 1) * QC],
                        start=(ks == 0),
                        stop=(ks == 1),
                    )
                    if after is not None and ks == 0:
                        # pin behind the gating attention matmul so the
                        # static scheduler doesn't hoist the projection
                        # ahead of the (slow) normalization chain
                        bass._add_dep_helper(
                            mm.ins, after.ins, sync=False, reason="defer proj"
                        )
                ysb = outp.tile([P, QC], BF16, tag="y", name=f"y_{m}_{nn}")
                nc.vector.tensor_copy(ysb[:], ps[:])
                nc.sync.dma_start(y_r[:, m, nn * QC:(nn + 1) * QC], ysb[:])

            # ------------- qkv projections + interleaved attention -----------
            with tc.tile_pool(name="xpool", bufs=1) as xpool:
                # load xt in COLUMN chunks (all 8 k-planes per 512-col range):
                # kt/qt chunk c and v tiles t=4c..4c+3 consume only columns
                # [512c, 512c+512), so the first attention work starts after
                # 1MB instead of the full 4MB transfer
                xt_sb = xpool.tile([P, KD, N], BF16, tag="xt")
                nc.sync.dma_start(xt_sb[:, :, 0:QC], xt_r[:, :, 0:QC])
                nc.sync.dma_start(wq_sb[:], wq_r)
                nc.sync.dma_start(xt_sb[:, :, QC:2 * QC], xt_r[:, :, QC:2 * QC])
                nc.sync.dma_start(wv_sb[:], wv_r)
                nc.sync.dma_start(
                    xt_sb[:, :, 2 * QC:3 * QC], xt_r[:, :, 2 * QC:3 * QC]
                )
                nc.sync.dma_start(wo_sb[:], wo_r)
                nc.sync.dma_start(
                    xt_sb[:, :, 3 * QC:4 * QC], xt_r[:, :, 3 * QC:4 * QC]
                )

                def emit_kq_group(which, w_sb, dst, m, c):
                    ps = ps_a.tile([P, QC], F32, tag="a", name=f"{which}ps_{m}_{c}")
                    for k in range(KD):
                        nc.tensor.matmul(
                            ps[:],
                            w_sb[:, k, m * P:(m + 1) * P],
                            xt_sb[:, k, c * QC:(c + 1) * QC],
                            start=(k == 0),
                            stop=(k == KD - 1),
                        )
                    nc.vector.tensor_copy(dst[:, m, c * QC:(c + 1) * QC], ps[:])

                # minimal prefix: kT m0c0 + qT m0c0 — everything else is
                # emitted just-in-time inside the attention stream so ScalarE
                # (the bottleneck engine) saturates as early as possible
                emit_kq_group("k", wk_sb, kt_sb, 0, 0)
                emit_kq_group("q", wq_sb, qt_sb, 0, 0)

                def emit_v(t):
                    # v = x @ wv -> vg_sb[t] per-head (interleaved in block 0)
                    ps = ps_a.tile([P, QC], F32, tag="a", name=f"vps_{t}")
                    for k in range(KD):
                        nc.tensor.matmul(
                            ps[:, :DG],
                            xt_sb[:, k, t * P:(t + 1) * P],
                            wv_sb[:, k, :],
                            start=(k == 0),
                            stop=(k == KD - 1),
                        )
                    nc.vector.tensor_copy(
                        vg_sb[:, t, :, 0:HD],
                        ps[:, :DG].rearrange("p (h e) -> p h e", h=GH),
                    )

                # per-block {t: group} JIT emissions: block (0,0) computes the
                # kt groups it consumes itself plus kt m1 c0 / qt m1 c0 for
                # (0,1); (0,1) computes its own later kt m1 chunks; each block
                # emits the q chunk needed two blocks ahead
                kq_jit = {
                    (0, 0): {
                        0: ("k", wk_sb, kt_sb, 0, 1),
                        1: ("k", wk_sb, kt_sb, 0, 2),
                        2: ("k", wk_sb, kt_sb, 0, 3),
                        5: ("k", wk_sb, kt_sb, 1, 0),
                        13: ("q", wq_sb, qt_sb, 1, 0),
                    },
                    (0, 1): {
                        0: ("k", wk_sb, kt_sb, 1, 1),
                        3: ("q", wq_sb, qt_sb, 0, 1),
                        4: ("k", wk_sb, kt_sb, 1, 2),
                        9: ("k", wk_sb, kt_sb, 1, 3),
                    },
                    (1, 0): {3: ("q", wq_sb, qt_sb, 1, 1)},
                    (1, 1): {3: ("q", wq_sb, qt_sb, 0, 2)},
                    (2, 0): {3: ("q", wq_sb, qt_sb, 1, 2)},
                    (2, 1): {3: ("q", wq_sb, qt_sb, 0, 3)},
                    (3, 0): {3: ("q", wq_sb, qt_sb, 1, 3)},
                }

                # ---- flat stream over all (block, t) steps with the PV
                # matmuls software-pipelined THREE iterations behind their
                # scores/exp: by the time PV(t) is issued, ACT(t) has long
                # finished, so the in-order PE queue never stalls on ScalarE
                # and the LDWEIGHTS prefetch stays hidden.  At each block
                # start the previous block's remaining PVs flush as a burst
                # (covered by ACT(B,0)) so its epilogue chain starts early.
                LAG = 3
                pending_proj = []
                pv_q = []
                epi_q = None
                last_gate = [None]

                def pop_pv():
                    last_gate[0] = emit_pv(*pv_q.pop(0))

                for c, pr in [(0, 0), (0, 1), (1, 0), (1, 1),
                              (2, 0), (2, 1), (3, 0), (3, 1)]:
                    o_ps = alloc_o(c, pr)
                    jit = kq_jit.get((c, pr), {})
                    for t in range(NT):
                        if (c, pr) == (0, 0):
                            emit_v(t)
                        e = emit_scores_act(c, pr, t)
                        if len(pv_q) >= LAG:
                            pop_pv()
                        pv_q.append((c, pr, t, e, o_ps))
                        if t == 0 and epi_q is not None:
                            while len(pv_q) > 1:
                                pop_pv()
                            emit_epilogue(*epi_q, split=2)
                            epi_q = None
                        if t in jit:
                            emit_kq_group(*jit[t])
                        # (3,1) keeps 2 chunk-2 units back to fill the PE
                        # during the drain epilogue chain (avoids the >3.4us
                        # PE idle that drops the HAM clock to 1.2GHz)
                        slots = (5,) if (c, pr) == (3, 1) else (5, 7, 9, 11, 13)
                        if pending_proj and t in slots:
                            emit_proj_unit(pending_proj.pop(0), after=last_gate[0])
                    epi_q = (c, pr, o_ps)
                    if pr == 1:
                        pending_proj.extend(
                            (4 * c + mi, nn) for mi in range(4) for nn in range(2)
                        )

                # drain: remaining PVs, held-back chunk-2 units (PE filler
                # under the final epilogue chain), then the final epilogue
                # with chunk-3 proj units interleaved between its splits
                # (split s covers m-tiles 12+2s..13+2s)
                while pv_q:
                    pop_pv()
                for unit in pending_proj:
                    if unit[0] < 12:
                        emit_proj_unit(unit)

                def tail_proj(s):
                    for mi in (12 + 2 * s, 13 + 2 * s):
                        for nn in range(2):
                            emit_proj_unit((mi, nn))

                emit_epilogue(*epi_q, split=2, after_split=tail_proj)

    nc.finalize()
    return nc


_NC = None


def _get_nc():
    global _NC
    if _NC is None:
        _NC = build_nc()
    return _NC


def _in_maps(x, w_qkv, w_out):
    bf = ml_dtypes.bfloat16
    x = np.asarray(x, dtype=np.float32)
    w_qkv = np.asarray(w_qkv, dtype=np.float32)
    w_out = np.asarray(w_out, dtype=np.float32)
    xts = [np.ascontiguousarray(x[b].T).astype(bf) for b in range(2)]
    wq_g = [np.ascontiguousarray(w_qkv[:, 0 * D + g * DG:0 * D + (g + 1) * DG]).astype(bf) for g in range(4)]
    wk_g = [np.ascontiguousarray(w_qkv[:, 1 * D + g * DG:1 * D + (g + 1) * DG]).astype(bf) for g in range(4)]
    wv_g = [np.ascontiguousarray(w_qkv[:, 2 * D + g * DG:2 * D + (g + 1) * DG]).astype(bf) for g in range(4)]
    wo_g = [np.ascontiguousarray(w_out[g * DG:(g + 1) * DG, :]).astype(bf) for g in range(4)]
    maps = []
    for c in range(8):
        b, g = c // 4, c % 4
        maps.append({
            "xt": xts[b],
            "wq": wq_g[g],
            "wk": wk_g[g],
            "wv": wv_g[g],
            "wo": wo_g[g],
        })
    return maps


LAST_RESULT = None


def kernel(x, w_qkv, w_out, b_out):
    from concourse.bass_utils import run_bass_kernel_spmd

    nc = _get_nc()
    maps = _in_maps(x, w_qkv, w_out)
    res = run_bass_kernel_spmd(nc, maps, list(range(8)))
    global LAST_RESULT
    LAST_RESULT = res
    out = np.zeros((2, N, D), dtype=np.float32)
    for c in range(8):
        out[c // 4] += np.asarray(res.results[c]["y"], dtype=np.float32)
    out += np.asarray(b_out, dtype=np.float32)[None, None, :]
    return out

